# revision 17
# baseline (speedup 1.0000x reference)
"""Conv2D 3x3 stride-1 pad-1 (NCHW) as implicit GEMM on 8 NeuronCores.

Strategy: data-parallel over batch (32 imgs -> 4 per core). The input is
zero-padded on the host to (*, 128, 58, 58) so each image DMAs contiguously
into an SBUF tile [C=128, 58, 58] with input channels on partitions.
Weights are preprocessed host-side to [I=128, (kh kw o)] so each
(tap, ochunk) slice is a ready [K=128, M=128] stationary operand.
Output rows are processed in groups of 8 (moving free dim N = 8*56 = 448).

bf16v2 layout decisions (vs the older modes kept below for A/B):
- bf16 operands: FWL makes LDWEIGHTS (~100ns) hide under the 192ns matmul
  stream, and x/w DMA bytes halve. rel-err ~2e-3, well under the 2e-2 gate.
- group-major loop (9 taps accumulate one PSUM group back-to-back): each
  group's output is ready every ~1.7us instead of all 7 at the end of a
  pass, so bias-add + store-DMA load is smooth and the kernel tail is one
  group, not seven. It also means image-0 compute can start once 10 padded
  rows have landed.
- bf16 device output (host converts back to f32): halves store traffic.
- store DMAs alternate the sync/scalar HWDGE queues; image prefetch is
  split across the gpsimd/vector queues; weights stream on scalar. A
  single dma_start tops out ~155 GB/s and costs ~0.6us of issue time on
  its queue, so spreading queues is what keeps DMA off the critical path.

x (4,128,58,58) bf16 -> out (4,256,56,56) bf16 per core; no collectives.
"""

import os
import sys

import numpy as np

if "/opt/trn_rl_repo" not in sys.path:
    sys.path.insert(0, "/opt/trn_rl_repo")

from concourse import bacc, bass, mybir  # noqa: E402
from concourse.bass_utils import run_bass_kernel_spmd  # noqa: E402
from concourse.tile import TileContext, add_dep_helper  # noqa: E402

N_FULL, CIN, H, W = 32, 128, 56, 56
COUT = 256
KH = KW = 3
NCORES = 8
NPER = N_FULL // NCORES  # 4 images per core
HP, WP = H + 2, W + 2  # 58 x 58 padded
ROWS = 8  # output rows per matmul group
NFREE = ROWS * W  # 448 moving free dim (<= 512 for one PSUM bank)
NGROUPS = H // ROWS  # 7
OCH = COUT // 128  # 2 output-channel chunks

MODE = os.environ.get("CONV_MM_MODE", "bf16v2")

_CACHE = {}


def _build_conv_v2():
    f32 = mybir.dt.float32
    bf16 = mybir.dt.bfloat16

    # Bacc (not raw Bass): its compile pipeline legalizes sync waits --
    # TRN2 instructions carry at most one wait slot.
    nc = bacc.Bacc(None, target_bir_lowering=False)

    x_par = nc.declare_dram_parameter("x", [NPER, CIN, HP, WP], bf16, isOutput=False)
    # oc-chunk-major weight layout [CIN, (oc tap m)]: every head chunk is a
    # contiguous multi-KB-per-partition line (the [p, (tap o)] layout made
    # each slice a 256B/partition pattern that crawled at ~100 GB/s).
    w_par = nc.declare_dram_parameter(
        "wt", [CIN, OCH * KH * KW * 128], bf16, isOutput=False
    )
    bias_par = nc.declare_dram_parameter("bias", [COUT], f32, isOutput=False)
    out_par = nc.declare_dram_parameter("out", [NPER, COUT, H, W], bf16, isOutput=True)
    out_flat = out_par.rearrange("n o h w -> n o (h w)")

    with TileContext(nc) as tc:
        with (
            tc.tile_pool(name="const", bufs=1) as cpool,
            tc.tile_pool(name="psum", bufs=8, space="PSUM") as ppool,
            tc.tile_pool(name="outp", bufs=4) as opool,
        ):
            # HAM pre-warm: junk matmuls with NO upstream dependency. The
            # memset writer comes AFTER them in program order, so the tile
            # is legally written (the allocator requires a writer) but the
            # matmuls get only a harmless WAR edge and start the moment the
            # PE queue opens (~6.6us) instead of waiting ~1.4us for a
            # gpsimd memset semaphore. Garbage operands are fine: results
            # land in a PSUM bank that the first real start=True matmul
            # clears. 10 matmuls ~= 3.7us busy >= the 3.4us HAM window, so
            # the 8/8 (2.4 GHz) clock flip lands before the real stream.
            jnk = cpool.tile([128, 512], f32, tag="jnk")
            jnk_mm = jnk.bitcast(bf16)
            ps_jnk = ppool.tile([128, NFREE], f32, tag="ps", name="ps")
            for _ in range(10):
                nc.tensor.matmul(
                    ps_jnk[:],
                    jnk_mm[:, 0:128],
                    jnk_mm[:, 0:NFREE],
                    start=True,
                    stop=True,
                )

            # Two padded-x buffers (double buffering across images); the zero
            # borders come in with the host-padded DMA.
            xpads = [
                cpool.tile([CIN, HP, WP], bf16, tag=f"xpad{b}", name="xpad")
                for b in range(2)
            ]
            w_sb = cpool.tile([CIN, OCH * KH * KW * 128], bf16, tag="w", name="w")

            # Head loads. The first matmul is gated on x rows [0,10)
            # (gpsimd SWDGE: that queue opens earliest and its completion
            # sem lands sooner than the HWDGE receipt path) + weight taps
            # 0:6 of oc0 (scalar queue); later chunks stream in FIFO
            # behind them, always ahead of group-major consumption.
            nc.gpsimd.dma_start(out=xpads[0][:, 0:10, :], in_=x_par[0][:, 0:10, :])
            nc.scalar.dma_start(out=w_sb[:, 0:768], in_=w_par[:, 0:768])
            nc.sync.dma_start(out=xpads[0][:, 10:18, :], in_=x_par[0][:, 10:18, :])
            nc.scalar.dma_start(out=w_sb[:, 768:1152], in_=w_par[:, 768:1152])
            nc.sync.dma_start(out=xpads[0][:, 18:34, :], in_=x_par[0][:, 18:34, :])
            nc.scalar.dma_start(out=w_sb[:, 1152:2304], in_=w_par[:, 1152:2304])
            bias_sb = cpool.tile([128, OCH], f32, tag="bias")
            nc.gpsimd.dma_start(
                out=bias_sb[:], in_=bias_par.rearrange("(a b) -> b a", b=128)
            )
            nc.sync.dma_start(out=xpads[0][:, 34:HP, :], in_=x_par[0][:, 34:HP, :])

            mm_first = None
            x1_dmas = []  # image-1 loads, deferred until the first matmul
            out_q = [nc.sync, nc.scalar]
            qi = 0
            for n in range(NPER):
                xt = xpads[n % 2]
                if n >= 1:
                    # Image n streams into its buffer while image n-1
                    # computes; tile's WAR deps hold the DMA until image n-2
                    # released the buffer. SWDGE (gpsimd) queue so the
                    # ~5us transfer never blocks the HWDGE store rings.
                    d0 = nc.gpsimd.dma_start(out=xt[:], in_=x_par[n])
                    if n == 1:
                        x1_dmas.append(d0)
                for oc in range(OCH):
                    for g in range(NGROUPS):
                        ps = ppool.tile([128, NFREE], f32, tag="ps", name="ps")
                        for tap in range(KH * KW):
                            kh, kw = divmod(tap, KW)
                            wof = (oc * KH * KW + tap) * 128
                            lhsT = w_sb[:, wof : wof + 128]
                            mm = nc.tensor.matmul(
                                ps[:],
                                lhsT,
                                xt[:, g * ROWS + kh : g * ROWS + kh + ROWS, kw : kw + W],
                                start=(tap == 0),
                                stop=(tap == KH * KW - 1),
                            )
                            if mm_first is None:
                                mm_first = mm
                        ot = opool.tile([128, NFREE], bf16, tag="ot", name="ot")
                        nc.vector.tensor_scalar_add(
                            out=ot[:], in0=ps[:], scalar1=bias_sb[:, oc : oc + 1]
                        )
                        q = out_q[qi % 2]
                        qi += 1
                        q.dma_start(
                            out=out_flat[
                                n,
                                oc * 128 : (oc + 1) * 128,
                                g * NFREE : (g + 1) * NFREE,
                            ],
                            in_=ot[:],
                        )
            # The warmup tile's writer, placed LAST: it satisfies the
            # allocator's write-before-read rule for jnk while giving the
            # junk matmuls zero upstream deps, and sitting at the end of
            # the gpsimd program it no longer blocks that queue (its WAR
            # on the junk matmuls would otherwise stall bias/image issues
            # behind it until ~10us).
            nc.gpsimd.memset(jnk[:], 1.0)
            for d in x1_dmas:
                add_dep_helper(
                    d.ins, mm_first.ins, sync=True, reason="defer image-1 load"
                )
    nc.compile()
    return nc


def _build_conv(mode):
    if mode == "bf16v2":
        return _build_conv_v2()
    f32 = mybir.dt.float32
    bf16 = mybir.dt.bfloat16
    if mode == "fp32":
        mm_dt, io_dt = f32, f32
    elif mode == "fp32r":
        mm_dt, io_dt = mybir.dt.float32r, f32
    elif mode in ("bf16", "split3"):
        mm_dt, io_dt = bf16, bf16
    else:
        raise ValueError(mode)

    nc = bacc.Bacc(None, target_bir_lowering=False)

    if mode == "split3":
        x_names = ["xh", "xl"]
        w_names = ["wh", "wl"]
        terms = [(0, 0), (0, 1), (1, 0)]
    else:
        x_names = ["x"]
        w_names = ["wt"]
        terms = [(0, 0)]

    x_par = [
        nc.declare_dram_parameter(nm, [NPER, CIN, HP, WP], io_dt, isOutput=False)
        for nm in x_names
    ]
    w_par = [
        nc.declare_dram_parameter(nm, [CIN, KH * KW * COUT], io_dt, isOutput=False)
        for nm in w_names
    ]
    bias_par = nc.declare_dram_parameter("bias", [COUT], f32, isOutput=False)
    out_par = nc.declare_dram_parameter("out", [NPER, COUT, H, W], f32, isOutput=True)
    out_flat = out_par.rearrange("n o h w -> n o (h w)")

    def mmview(ap):
        return ap.bitcast(mm_dt) if mm_dt != io_dt else ap

    nmm_per_psum = KH * KW * len(terms)

    with TileContext(nc) as tc:
        with (
            tc.tile_pool(name="const", bufs=1) as cpool,
            tc.tile_pool(name="xpad", bufs=1) as xpool,
            tc.tile_pool(name="psum", bufs=8, space="PSUM") as ppool,
            tc.tile_pool(name="outp", bufs=4) as opool,
        ):
            jnk = cpool.tile([128, 512], f32, tag="jnk")
            nc.vector.memset(jnk[:], 1.0)
            jnk_mm = jnk if mm_dt == f32 else jnk.bitcast(mm_dt)
            ps_jnk = ppool.tile([128, NFREE], f32, tag="ps", name="ps")
            for _ in range(8):
                nc.tensor.matmul(
                    ps_jnk[:],
                    jnk_mm[:, 0:128],
                    jnk_mm[:, 0:NFREE],
                    start=True,
                    stop=True,
                )

            xpads = []
            for b in range(2):
                per_buf = []
                for xi in range(len(x_par)):
                    t = xpool.tile(
                        [CIN, HP, WP], mm_dt, tag=f"xpad{b}_{xi}", name="xpad"
                    )
                    per_buf.append(t)
                xpads.append(per_buf)

            XSPLIT = 34
            w_sb = []
            w3s = []
            for wi, wp in enumerate(w_par):
                t = cpool.tile([CIN, KH * KW * COUT], mm_dt, tag=f"w{wi}", name="w")
                w_sb.append(t)
                w3s.append(
                    (
                        t.rearrange("p (t o) -> p t o", t=KH * KW),
                        mmview(wp[:]).rearrange("p (t o) -> p t o", t=KH * KW),
                    )
                )
            for xi, xp in enumerate(x_par):
                nc.sync.dma_start(
                    out=xpads[0][xi][:, 0:17, :], in_=mmview(xp[0])[:, 0:17, :]
                )
            for t3, w3 in w3s:
                nc.sync.dma_start(out=t3[:, 0:5, 0:128], in_=w3[:, 0:5, 0:128])
            for xi, xp in enumerate(x_par):
                nc.sync.dma_start(
                    out=xpads[0][xi][:, 17:XSPLIT, :],
                    in_=mmview(xp[0])[:, 17:XSPLIT, :],
                )
            for t3, w3 in w3s:
                nc.sync.dma_start(out=t3[:, 5:9, 0:128], in_=w3[:, 5:9, 0:128])
            bias_sb = cpool.tile([128, OCH], f32, tag="bias")
            nc.sync.dma_start(
                out=bias_sb[:], in_=bias_par.rearrange("(a b) -> b a", b=128)
            )
            tail_dmas = []
            for xi, xp in enumerate(x_par):
                d = nc.sync.dma_start(
                    out=xpads[0][xi][:, XSPLIT:HP, :],
                    in_=mmview(xp[0])[:, XSPLIT:HP, :],
                )
                tail_dmas.append(d)
            for t3, w3 in w3s:
                d = nc.sync.dma_start(out=t3[:, :, 128:256], in_=w3[:, :, 128:256])
                tail_dmas.append(d)

            mm_first = None
            mm_oc1_first = None
            x1_dmas = []
            for n in range(NPER):
                bufs = xpads[n % 2]
                if n >= 1:
                    for xi, xp in enumerate(x_par):
                        d = nc.gpsimd.dma_start(out=bufs[xi][:], in_=mmview(xp[n]))
                        if n == 1:
                            x1_dmas.append(d)
                for oc in range(OCH):
                    psums = [
                        ppool.tile([128, NFREE], f32, tag="ps", name="ps")
                        for _ in range(NGROUPS)
                    ]
                    i_mm = 0
                    for xi, wi in terms:
                        xt = bufs[xi]
                        for tap in range(KH * KW):
                            kh, kw = divmod(tap, KW)
                            lhsT = w_sb[wi][
                                :, tap * COUT + oc * 128 : tap * COUT + oc * 128 + 128
                            ]
                            for g in range(NGROUPS):
                                mm = nc.tensor.matmul(
                                    psums[g][:],
                                    lhsT,
                                    xt[
                                        :,
                                        g * ROWS + kh : g * ROWS + kh + ROWS,
                                        kw : kw + W,
                                    ],
                                    start=(i_mm == 0),
                                    stop=(i_mm == nmm_per_psum - 1),
                                )
                                if n == 0 and i_mm == 0 and g == 0:
                                    if oc == 0:
                                        mm_first = mm
                                    else:
                                        mm_oc1_first = mm
                            i_mm += 1
                    for g in range(NGROUPS):
                        ot = opool.tile([128, NFREE], f32, tag="ot", name="ot")
                        nc.vector.tensor_scalar_add(
                            out=ot[:], in0=psums[g][:], scalar1=bias_sb[:, oc : oc + 1]
                        )
                        nc.sync.dma_start(
                            out=out_flat[
                                n,
                                oc * 128 : (oc + 1) * 128,
                                g * NFREE : (g + 1) * NFREE,
                            ],
                            in_=ot[:],
                        )
            for d in tail_dmas:
                add_dep_helper(
                    d.ins, mm_first.ins, sync=True, reason="defer past first matmul"
                )
            for d in x1_dmas:
                add_dep_helper(
                    d.ins, mm_oc1_first.ins, sync=True, reason="defer image-1 load"
                )
    nc.compile()
    return nc


def _get_nc(mode):
    if mode not in _CACHE:
        _CACHE[mode] = _build_conv(mode)
    return _CACHE[mode]


# test-harness hooks: set TRACE=True before calling kernel() to capture an
# NTFF profile; LAST_RESULTS then holds the BassKernelResults.
TRACE = False
LAST_RESULTS = None


def kernel(x, weight, bias):
    global LAST_RESULTS
    mode = MODE
    x = np.ascontiguousarray(np.asarray(x), dtype=np.float32)
    w = np.ascontiguousarray(np.asarray(weight), dtype=np.float32)
    b = np.ascontiguousarray(np.asarray(bias), dtype=np.float32)
    xp = np.pad(x, ((0, 0), (0, 0), (1, 1), (1, 1)))
    # wt[i, (kh kw o)] = w[o, i, kh, kw]
    wt = np.ascontiguousarray(w.transpose(1, 2, 3, 0).reshape(CIN, KH * KW * COUT))

    if mode in ("fp32", "fp32r"):
        per_core = [
            {"x": xp[c * NPER : (c + 1) * NPER], "wt": wt, "bias": b}
            for c in range(NCORES)
        ]
    else:
        import ml_dtypes

        bfl = ml_dtypes.bfloat16
        if mode in ("bf16", "bf16v2"):
            xh = xp.astype(bfl)
            if mode == "bf16v2":
                # wt2[i, (oc tap m)] = w[oc*128+m, i, kh, kw]: contiguous
                # per-partition lines for each (oc, tap-range) head chunk.
                wt2 = np.ascontiguousarray(
                    w.reshape(OCH, 128, CIN, KH * KW)
                    .transpose(2, 0, 3, 1)
                    .reshape(CIN, OCH * KH * KW * 128)
                )
                wth = wt2.astype(bfl)
            else:
                wth = wt.astype(bfl)
            per_core = [
                {"x": xh[c * NPER : (c + 1) * NPER], "wt": wth, "bias": b}
                for c in range(NCORES)
            ]
        else:  # split3
            xh = xp.astype(bfl)
            xl = (xp - xh.astype(np.float32)).astype(bfl)
            wh = wt.astype(bfl)
            wl = (wt - wh.astype(np.float32)).astype(bfl)
            per_core = [
                {
                    "xh": xh[c * NPER : (c + 1) * NPER],
                    "xl": xl[c * NPER : (c + 1) * NPER],
                    "wh": wh,
                    "wl": wl,
                    "bias": b,
                }
                for c in range(NCORES)
            ]

    kwargs = {}
    if TRACE:
        kwargs = dict(trace=True, trace_cores=[0])
    res = run_bass_kernel_spmd(
        _get_nc(mode), per_core, core_ids=list(range(NCORES)), **kwargs
    )
    LAST_RESULTS = res
    out = np.concatenate([r["out"] for r in res.results], axis=0)
    if out.dtype != np.float32:
        out = out.astype(np.float32)
    return out


# revision 20
# speedup vs baseline: 1.0198x; 1.0198x over previous
"""Conv2D 3x3 stride-1 pad-1 (NCHW) as implicit GEMM on 8 NeuronCores.

Strategy: data-parallel over batch (32 imgs -> 4 per core). The input is
zero-padded on the host to (*, 128, 58, 58) so each image DMAs contiguously
into an SBUF tile [C=128, 58, 58] with input channels on partitions.
Weights are preprocessed host-side to [I=128, (kh kw o)] so each
(tap, ochunk) slice is a ready [K=128, M=128] stationary operand.
Output rows are processed in groups of 8 (moving free dim N = 8*56 = 448).

bf16v2 layout decisions (vs the older modes kept below for A/B):
- bf16 operands: FWL makes LDWEIGHTS (~100ns) hide under the 192ns matmul
  stream, and x/w DMA bytes halve. rel-err ~2e-3, well under the 2e-2 gate.
- group-major loop (9 taps accumulate one PSUM group back-to-back): each
  group's output is ready every ~1.7us instead of all 7 at the end of a
  pass, so bias-add + store-DMA load is smooth and the kernel tail is one
  group, not seven. It also means image-0 compute can start once 10 padded
  rows have landed.
- bf16 device output (host converts back to f32): halves store traffic.
- store DMAs alternate the sync/scalar HWDGE queues; image prefetch is
  split across the gpsimd/vector queues; weights stream on scalar. A
  single dma_start tops out ~155 GB/s and costs ~0.6us of issue time on
  its queue, so spreading queues is what keeps DMA off the critical path.

x (4,128,58,58) bf16 -> out (4,256,56,56) bf16 per core; no collectives.
"""

import os
import sys

import numpy as np

if "/opt/trn_rl_repo" not in sys.path:
    sys.path.insert(0, "/opt/trn_rl_repo")

from concourse import bacc, bass, mybir  # noqa: E402
from concourse.bass_utils import run_bass_kernel_spmd  # noqa: E402
from concourse.tile import TileContext, add_dep_helper  # noqa: E402

N_FULL, CIN, H, W = 32, 128, 56, 56
COUT = 256
KH = KW = 3
NCORES = 8
NPER = N_FULL // NCORES  # 4 images per core
HP, WP = H + 2, W + 2  # 58 x 58 padded
ROWS = 8  # output rows per matmul group
NFREE = ROWS * W  # 448 moving free dim (<= 512 for one PSUM bank)
NGROUPS = H // ROWS  # 7
OCH = COUT // 128  # 2 output-channel chunks

MODE = os.environ.get("CONV_MM_MODE", "bf16v2")

_CACHE = {}


def _build_conv_v2():
    f32 = mybir.dt.float32
    bf16 = mybir.dt.bfloat16

    # Bacc (not raw Bass): its compile pipeline legalizes sync waits --
    # TRN2 instructions carry at most one wait slot.
    nc = bacc.Bacc(None, target_bir_lowering=False)

    x_par = nc.declare_dram_parameter("x", [NPER, CIN, HP, WP], bf16, isOutput=False)
    # oc-chunk-major weight layout [CIN, (oc tap m)]: every head chunk is a
    # contiguous multi-KB-per-partition line (the [p, (tap o)] layout made
    # each slice a 256B/partition pattern that crawled at ~100 GB/s).
    w_par = nc.declare_dram_parameter(
        "wt", [CIN, OCH * KH * KW * 128], bf16, isOutput=False
    )
    bias_par = nc.declare_dram_parameter("bias", [COUT], f32, isOutput=False)
    out_par = nc.declare_dram_parameter("out", [NPER, COUT, H, W], bf16, isOutput=True)
    out_flat = out_par.rearrange("n o h w -> n o (h w)")

    with TileContext(nc) as tc:
        with (
            tc.tile_pool(name="const", bufs=1) as cpool,
            tc.tile_pool(name="psum", bufs=8, space="PSUM") as ppool,
            tc.tile_pool(name="outp", bufs=4) as opool,
        ):
            # HAM pre-warm: junk matmuls with NO upstream dependency. The
            # memset writer comes AFTER them in program order, so the tile
            # is legally written (the allocator requires a writer) but the
            # matmuls get only a harmless WAR edge and start the moment the
            # PE queue opens (~6.6us) instead of waiting ~1.4us for a
            # gpsimd memset semaphore. Garbage operands are fine: results
            # land in a PSUM bank that the first real start=True matmul
            # clears. 10 matmuls ~= 3.7us busy >= the 3.4us HAM window, so
            # the 8/8 (2.4 GHz) clock flip lands before the real stream.
            jnk = cpool.tile([128, 512], f32, tag="jnk")
            jnk_mm = jnk.bitcast(bf16)
            ps_jnk = ppool.tile([128, NFREE], f32, tag="ps", name="ps")
            for _ in range(10):
                nc.tensor.matmul(
                    ps_jnk[:],
                    jnk_mm[:, 0:128],
                    jnk_mm[:, 0:NFREE],
                    start=True,
                    stop=True,
                )
            nc.gpsimd.memset(jnk[:], 1.0)

            # Two padded-x buffers (double buffering across images); the zero
            # borders come in with the host-padded DMA.
            xpads = [
                cpool.tile([CIN, HP, WP], bf16, tag=f"xpad{b}", name="xpad")
                for b in range(2)
            ]
            w_sb = cpool.tile([CIN, OCH * KH * KW * 128], bf16, tag="w", name="w")

            # Head loads. The first matmul is gated on x rows [0,10) (sync
            # queue) + weight taps 0:6 of oc0 (scalar queue); later chunks
            # stream in FIFO behind them on their queues, always ahead of
            # the group-major consumption order. (Tried and reverted:
            # gating x[0,10) via gpsimd SWDGE — its Q7 emission+receipt
            # path fired the sem ~2.5us LATER than the sync HWDGE ring.)
            nc.sync.dma_start(out=xpads[0][:, 0:10, :], in_=x_par[0][:, 0:10, :])
            nc.scalar.dma_start(out=w_sb[:, 0:768], in_=w_par[:, 0:768])
            nc.sync.dma_start(out=xpads[0][:, 10:18, :], in_=x_par[0][:, 10:18, :])
            nc.scalar.dma_start(out=w_sb[:, 768:1152], in_=w_par[:, 768:1152])
            nc.sync.dma_start(out=xpads[0][:, 18:34, :], in_=x_par[0][:, 18:34, :])
            nc.scalar.dma_start(out=w_sb[:, 1152:2304], in_=w_par[:, 1152:2304])
            bias_sb = cpool.tile([128, OCH], f32, tag="bias")
            nc.gpsimd.dma_start(
                out=bias_sb[:], in_=bias_par.rearrange("(a b) -> b a", b=128)
            )
            nc.sync.dma_start(out=xpads[0][:, 34:HP, :], in_=x_par[0][:, 34:HP, :])

            mm_first = None
            x1_dmas = []  # image-1 loads, deferred until the first matmul
            out_q = [nc.sync, nc.scalar]
            qi = 0
            for n in range(NPER):
                xt = xpads[n % 2]
                if n >= 1:
                    # Image n streams into its buffer while image n-1
                    # computes; tile's WAR deps hold the DMA until image n-2
                    # released the buffer. SWDGE (gpsimd) queue so the
                    # ~5us transfer never blocks the HWDGE store rings.
                    d0 = nc.gpsimd.dma_start(out=xt[:], in_=x_par[n])
                    if n == 1:
                        x1_dmas.append(d0)
                for oc in range(OCH):
                    for g in range(NGROUPS):
                        ps = ppool.tile([128, NFREE], f32, tag="ps", name="ps")
                        for tap in range(KH * KW):
                            kh, kw = divmod(tap, KW)
                            wof = (oc * KH * KW + tap) * 128
                            lhsT = w_sb[:, wof : wof + 128]
                            mm = nc.tensor.matmul(
                                ps[:],
                                lhsT,
                                xt[:, g * ROWS + kh : g * ROWS + kh + ROWS, kw : kw + W],
                                start=(tap == 0),
                                stop=(tap == KH * KW - 1),
                            )
                            if mm_first is None:
                                mm_first = mm
                        ot = opool.tile([128, NFREE], bf16, tag="ot", name="ot")
                        nc.vector.tensor_scalar_add(
                            out=ot[:], in0=ps[:], scalar1=bias_sb[:, oc : oc + 1]
                        )
                        q = out_q[qi % 2]
                        qi += 1
                        q.dma_start(
                            out=out_flat[
                                n,
                                oc * 128 : (oc + 1) * 128,
                                g * NFREE : (g + 1) * NFREE,
                            ],
                            in_=ot[:],
                        )
            for d in x1_dmas:
                add_dep_helper(
                    d.ins, mm_first.ins, sync=True, reason="defer image-1 load"
                )
    nc.compile()
    return nc


def _build_conv(mode):
    if mode == "bf16v2":
        return _build_conv_v2()
    f32 = mybir.dt.float32
    bf16 = mybir.dt.bfloat16
    if mode == "fp32":
        mm_dt, io_dt = f32, f32
    elif mode == "fp32r":
        mm_dt, io_dt = mybir.dt.float32r, f32
    elif mode in ("bf16", "split3"):
        mm_dt, io_dt = bf16, bf16
    else:
        raise ValueError(mode)

    nc = bacc.Bacc(None, target_bir_lowering=False)

    if mode == "split3":
        x_names = ["xh", "xl"]
        w_names = ["wh", "wl"]
        terms = [(0, 0), (0, 1), (1, 0)]
    else:
        x_names = ["x"]
        w_names = ["wt"]
        terms = [(0, 0)]

    x_par = [
        nc.declare_dram_parameter(nm, [NPER, CIN, HP, WP], io_dt, isOutput=False)
        for nm in x_names
    ]
    w_par = [
        nc.declare_dram_parameter(nm, [CIN, KH * KW * COUT], io_dt, isOutput=False)
        for nm in w_names
    ]
    bias_par = nc.declare_dram_parameter("bias", [COUT], f32, isOutput=False)
    out_par = nc.declare_dram_parameter("out", [NPER, COUT, H, W], f32, isOutput=True)
    out_flat = out_par.rearrange("n o h w -> n o (h w)")

    def mmview(ap):
        return ap.bitcast(mm_dt) if mm_dt != io_dt else ap

    nmm_per_psum = KH * KW * len(terms)

    with TileContext(nc) as tc:
        with (
            tc.tile_pool(name="const", bufs=1) as cpool,
            tc.tile_pool(name="xpad", bufs=1) as xpool,
            tc.tile_pool(name="psum", bufs=8, space="PSUM") as ppool,
            tc.tile_pool(name="outp", bufs=4) as opool,
        ):
            jnk = cpool.tile([128, 512], f32, tag="jnk")
            nc.vector.memset(jnk[:], 1.0)
            jnk_mm = jnk if mm_dt == f32 else jnk.bitcast(mm_dt)
            ps_jnk = ppool.tile([128, NFREE], f32, tag="ps", name="ps")
            for _ in range(8):
                nc.tensor.matmul(
                    ps_jnk[:],
                    jnk_mm[:, 0:128],
                    jnk_mm[:, 0:NFREE],
                    start=True,
                    stop=True,
                )

            xpads = []
            for b in range(2):
                per_buf = []
                for xi in range(len(x_par)):
                    t = xpool.tile(
                        [CIN, HP, WP], mm_dt, tag=f"xpad{b}_{xi}", name="xpad"
                    )
                    per_buf.append(t)
                xpads.append(per_buf)

            XSPLIT = 34
            w_sb = []
            w3s = []
            for wi, wp in enumerate(w_par):
                t = cpool.tile([CIN, KH * KW * COUT], mm_dt, tag=f"w{wi}", name="w")
                w_sb.append(t)
                w3s.append(
                    (
                        t.rearrange("p (t o) -> p t o", t=KH * KW),
                        mmview(wp[:]).rearrange("p (t o) -> p t o", t=KH * KW),
                    )
                )
            for xi, xp in enumerate(x_par):
                nc.sync.dma_start(
                    out=xpads[0][xi][:, 0:17, :], in_=mmview(xp[0])[:, 0:17, :]
                )
            for t3, w3 in w3s:
                nc.sync.dma_start(out=t3[:, 0:5, 0:128], in_=w3[:, 0:5, 0:128])
            for xi, xp in enumerate(x_par):
                nc.sync.dma_start(
                    out=xpads[0][xi][:, 17:XSPLIT, :],
                    in_=mmview(xp[0])[:, 17:XSPLIT, :],
                )
            for t3, w3 in w3s:
                nc.sync.dma_start(out=t3[:, 5:9, 0:128], in_=w3[:, 5:9, 0:128])
            bias_sb = cpool.tile([128, OCH], f32, tag="bias")
            nc.sync.dma_start(
                out=bias_sb[:], in_=bias_par.rearrange("(a b) -> b a", b=128)
            )
            tail_dmas = []
            for xi, xp in enumerate(x_par):
                d = nc.sync.dma_start(
                    out=xpads[0][xi][:, XSPLIT:HP, :],
                    in_=mmview(xp[0])[:, XSPLIT:HP, :],
                )
                tail_dmas.append(d)
            for t3, w3 in w3s:
                d = nc.sync.dma_start(out=t3[:, :, 128:256], in_=w3[:, :, 128:256])
                tail_dmas.append(d)

            mm_first = None
            mm_oc1_first = None
            x1_dmas = []
            for n in range(NPER):
                bufs = xpads[n % 2]
                if n >= 1:
                    for xi, xp in enumerate(x_par):
                        d = nc.gpsimd.dma_start(out=bufs[xi][:], in_=mmview(xp[n]))
                        if n == 1:
                            x1_dmas.append(d)
                for oc in range(OCH):
                    psums = [
                        ppool.tile([128, NFREE], f32, tag="ps", name="ps")
                        for _ in range(NGROUPS)
                    ]
                    i_mm = 0
                    for xi, wi in terms:
                        xt = bufs[xi]
                        for tap in range(KH * KW):
                            kh, kw = divmod(tap, KW)
                            lhsT = w_sb[wi][
                                :, tap * COUT + oc * 128 : tap * COUT + oc * 128 + 128
                            ]
                            for g in range(NGROUPS):
                                mm = nc.tensor.matmul(
                                    psums[g][:],
                                    lhsT,
                                    xt[
                                        :,
                                        g * ROWS + kh : g * ROWS + kh + ROWS,
                                        kw : kw + W,
                                    ],
                                    start=(i_mm == 0),
                                    stop=(i_mm == nmm_per_psum - 1),
                                )
                                if n == 0 and i_mm == 0 and g == 0:
                                    if oc == 0:
                                        mm_first = mm
                                    else:
                                        mm_oc1_first = mm
                            i_mm += 1
                    for g in range(NGROUPS):
                        ot = opool.tile([128, NFREE], f32, tag="ot", name="ot")
                        nc.vector.tensor_scalar_add(
                            out=ot[:], in0=psums[g][:], scalar1=bias_sb[:, oc : oc + 1]
                        )
                        nc.sync.dma_start(
                            out=out_flat[
                                n,
                                oc * 128 : (oc + 1) * 128,
                                g * NFREE : (g + 1) * NFREE,
                            ],
                            in_=ot[:],
                        )
            for d in tail_dmas:
                add_dep_helper(
                    d.ins, mm_first.ins, sync=True, reason="defer past first matmul"
                )
            for d in x1_dmas:
                add_dep_helper(
                    d.ins, mm_oc1_first.ins, sync=True, reason="defer image-1 load"
                )
    nc.compile()
    return nc


def _get_nc(mode):
    if mode not in _CACHE:
        _CACHE[mode] = _build_conv(mode)
    return _CACHE[mode]


# test-harness hooks: set TRACE=True before calling kernel() to capture an
# NTFF profile; LAST_RESULTS then holds the BassKernelResults.
TRACE = False
LAST_RESULTS = None


def kernel(x, weight, bias):
    global LAST_RESULTS
    mode = MODE
    x = np.ascontiguousarray(np.asarray(x), dtype=np.float32)
    w = np.ascontiguousarray(np.asarray(weight), dtype=np.float32)
    b = np.ascontiguousarray(np.asarray(bias), dtype=np.float32)
    xp = np.pad(x, ((0, 0), (0, 0), (1, 1), (1, 1)))
    # wt[i, (kh kw o)] = w[o, i, kh, kw]
    wt = np.ascontiguousarray(w.transpose(1, 2, 3, 0).reshape(CIN, KH * KW * COUT))

    if mode in ("fp32", "fp32r"):
        per_core = [
            {"x": xp[c * NPER : (c + 1) * NPER], "wt": wt, "bias": b}
            for c in range(NCORES)
        ]
    else:
        import ml_dtypes

        bfl = ml_dtypes.bfloat16
        if mode in ("bf16", "bf16v2"):
            xh = xp.astype(bfl)
            if mode == "bf16v2":
                # wt2[i, (oc tap m)] = w[oc*128+m, i, kh, kw]: contiguous
                # per-partition lines for each (oc, tap-range) head chunk.
                wt2 = np.ascontiguousarray(
                    w.reshape(OCH, 128, CIN, KH * KW)
                    .transpose(2, 0, 3, 1)
                    .reshape(CIN, OCH * KH * KW * 128)
                )
                wth = wt2.astype(bfl)
            else:
                wth = wt.astype(bfl)
            per_core = [
                {"x": xh[c * NPER : (c + 1) * NPER], "wt": wth, "bias": b}
                for c in range(NCORES)
            ]
        else:  # split3
            xh = xp.astype(bfl)
            xl = (xp - xh.astype(np.float32)).astype(bfl)
            wh = wt.astype(bfl)
            wl = (wt - wh.astype(np.float32)).astype(bfl)
            per_core = [
                {
                    "xh": xh[c * NPER : (c + 1) * NPER],
                    "xl": xl[c * NPER : (c + 1) * NPER],
                    "wh": wh,
                    "wl": wl,
                    "bias": b,
                }
                for c in range(NCORES)
            ]

    kwargs = {}
    if TRACE:
        kwargs = dict(trace=True, trace_cores=[0])
    res = run_bass_kernel_spmd(
        _get_nc(mode), per_core, core_ids=list(range(NCORES)), **kwargs
    )
    LAST_RESULTS = res
    out = np.concatenate([r["out"] for r in res.results], axis=0)
    if out.dtype != np.float32:
        out = out.astype(np.float32)
    return out
